# revision 10
# baseline (speedup 1.0000x reference)
"""Distributed Trainium2 Bass kernel for nn_Attention (dense transformer block).

Reference computation (full shapes):
    x: [2, 2048, 1024]
    xn = LayerNorm(x, gamma, beta)
    q = xn @ w_q ; k, v = split(xn @ w_kv)   (16 heads, head dim 64)
    attn = softmax(q k^T / 8) v  over seq 2048
    out = attn_out @ w_out + b_out           -> [2, 2048, 1024]

Sharding over 8 NeuronCores: ROW-sharded (i-sharding). Core c owns rows
[512c, 512c+512) of the flattened [4096, 1024] activations; batch = c//4,
so cores {0..3} / {4..7} form the two batch groups. Every core runs all 16
heads for its own 512 query rows. All 8 cores run one identical SPMD graph.

Per-core pipeline:
    1. LayerNorm own 512 rows -> xn^T [1024, 512] bf16 (PE transpose,
       gamma/beta fused on copyback)
    2. K-projection own rows (all heads) -> bounce -> AllGather-K over the
       4-core batch group (pipelined: scores need only K)
    3. V-projection -> bounce -> AllGather-V (runs behind AG-K)
    4. Q-projection own rows -> qT (local; overlaps AG-K)
    5. Attention per head-pair hp (2 heads = 128 ch): j over the full 2048
       batch rows from the gathered K/V; flash-style 128-row j-tiles:
       scores^T pair-packed in PE row groups, exp on ScalarE (x1/8 fused),
       attn@v accumulated transposed in PSUM with a ones-column block so
       softmax denominators come free; per-hp drain -> reciprocal ->
       normalize -> aoT
    6. Out-projection + bias on own 512 rows -> out [512, 1024] f32 (local,
       no second collective)

A tiny dummy AllGather fires first to pull the NRT collective-init barrier
as early as possible.
"""

import numpy as np

import concourse.bass as bass
import concourse.mybir as mybir
import concourse.tile as tile
from concourse import bacc
from concourse.bass_utils import run_bass_kernel_spmd

F32 = mybir.dt.float32
BF16 = mybir.dt.bfloat16
AF = mybir.ActivationFunctionType
ALU = mybir.AluOpType

N_CORES = 8
DIM = 1024
N = 2048  # sequence length
R = 4096  # total rows (2 batches x 2048)
RL = 512  # own rows per core
G = 4  # cores per batch group
HP = 8  # head-pairs (2 heads = 128 channels each)
DH = 64
KO = 8  # contraction chunks over DIM
NJT = 16  # j-tiles of 128 over the batch's 2048 rows
SCALE = DH**-0.5
GROUPS = [[0, 1, 2, 3], [4, 5, 6, 7]]


def build_nc():
    nc = bacc.Bacc("TRN2", target_bir_lowering=False, debug=False, num_devices=N_CORES)

    x_ext = nc.declare_dram_parameter("x", [RL, DIM], F32, isOutput=False)
    wq_ext = nc.declare_dram_parameter("wq", [DIM, DIM], F32, isOutput=False)
    wk_ext = nc.declare_dram_parameter("wk", [DIM, DIM], F32, isOutput=False)
    wv_ext = nc.declare_dram_parameter("wv", [DIM, DIM], F32, isOutput=False)
    wo_ext = nc.declare_dram_parameter("wo", [DIM, DIM], F32, isOutput=False)
    gamma_ext = nc.declare_dram_parameter("gamma", [DIM], F32, isOutput=False)
    beta_ext = nc.declare_dram_parameter("beta", [DIM], F32, isOutput=False)
    bias_ext = nc.declare_dram_parameter("bias", [DIM], F32, isOutput=False)
    out_ext = nc.declare_dram_parameter("out", [RL, DIM], F32, isOutput=True)

    # DRAM bounce buffers for collectives
    k_bounce = nc.dram_tensor("k_bounce", [DIM, RL], BF16)
    k_gath = nc.dram_tensor("k_gath", [G * DIM, RL], BF16)
    v_bounce = nc.dram_tensor("v_bounce", [RL, DIM], BF16)
    v_gath = nc.dram_tensor("v_gath", [G * RL, DIM], BF16)

    with tile.TileContext(nc) as tc:
        with (
            tc.tile_pool(name="singles", bufs=1) as singles,
            tc.tile_pool(name="wstage", bufs=2) as wstage,
            tc.tile_pool(name="temps", bufs=4) as temps,
            tc.tile_pool(name="small", bufs=4) as small,
            tc.tile_pool(name="kv", bufs=2) as kvpool,
            tc.tile_pool(name="etile", bufs=13) as epool,
            tc.tile_pool(name="psum", bufs=2, space="PSUM") as psum,
        ):
            # ---- constants ----
            import ml_dtypes

            ident_const = nc.inline_tensor(
                np.eye(128, dtype=ml_dtypes.bfloat16), name="ident_const"
            )
            ident = singles.tile([128, 128], BF16, tag="ident")
            nc.sync.dma_start(out=ident[:], in_=ident_const.ap())
            gamma_sb = singles.tile([128, KO], F32, tag="gamma")
            nc.sync.dma_start(
                out=gamma_sb[:], in_=gamma_ext.ap().rearrange("(ko p) -> p ko", p=128)
            )
            beta_sb = singles.tile([128, KO], F32, tag="beta")
            nc.sync.dma_start(
                out=beta_sb[:], in_=beta_ext.ap().rearrange("(ko p) -> p ko", p=128)
            )
            eps_sb = singles.tile([128, 1], F32, tag="eps")
            nc.vector.memset(eps_sb[:], 1e-5)
            bias_sb = singles.tile([128, DIM], F32, tag="bias")
            nc.sync.dma_start(
                out=bias_sb[:],
                in_=bass.AP(tensor=bias_ext, offset=0, ap=[[0, 128], [1, DIM]]),
            )

            # ---- input DMAs on the scalar queue: x first, then wk/wv/wq ----
            x_ts = []
            for t in range(4):
                x_t = temps.tile([128, DIM], F32, tag="x", name="x_t")
                nc.scalar.dma_start(out=x_t[:], in_=x_ext[t * 128 : (t + 1) * 128, :])
                x_ts.append(x_t)

            wk_b = singles.tile([128, KO, DIM], BF16, tag="wk")
            wv_b = singles.tile([128, KO, DIM], BF16, tag="wv")
            wq_b = singles.tile([128, KO, DIM], BF16, tag="wq")
            wo_b = singles.tile([128, KO, DIM], BF16, tag="wo")

            def stage_weight(ext, dst, cast_engine, dma_engine):
                # 8 chunks of 1 ko-slice ([128, 1024] f32), shares the x tag
                for c in range(KO):
                    wf = wstage.tile([128, DIM], F32, tag="wstage", name="wf")
                    dma_engine.dma_start(
                        out=wf[:], in_=ext[c * 128 : (c + 1) * 128, :]
                    )
                    if cast_engine is nc.scalar:
                        cast_engine.activation(
                            out=dst[:, c, :], in_=wf[:], func=AF.Copy
                        )
                    else:
                        cast_engine.tensor_copy(out=dst[:, c, :], in_=wf[:])

            stage_weight(wk_ext, wk_b, nc.scalar, nc.sync)
            stage_weight(wv_ext, wv_b, nc.scalar, nc.sync)
            stage_weight(wq_ext, wq_b, nc.vector, nc.sync)

            # ---- Phase 1: LayerNorm own rows + transpose ----
            xc = []
            for t in range(4):
                x_t = x_ts[t]
                st6 = small.tile([128, 2, 6], F32, tag="st6")
                nc.vector.bn_stats(out=st6[:, 0, :], in_=x_t[:, 0:512])
                nc.vector.bn_stats(out=st6[:, 1, :], in_=x_t[:, 512:1024])
                mv = small.tile([128, 2], F32, tag="mv")
                nc.vector.bn_aggr(out=mv[:], in_=st6[:])
                sd = small.tile([128, 1], F32, tag="sd")
                nc.scalar.activation(
                    out=sd[:], in_=mv[:, 1:2], func=AF.Sqrt, bias=eps_sb[:], scale=1.0
                )
                istd = small.tile([128, 1], F32, tag="istd")
                nc.vector.reciprocal(out=istd[:], in_=sd[:])
                xc_t = singles.tile([128, DIM], BF16, tag=f"xc{t}", name=f"xc{t}")
                for hh in range(2):
                    nc.vector.tensor_scalar(
                        xc_t[:, hh * 512 : (hh + 1) * 512],
                        x_t[:, hh * 512 : (hh + 1) * 512],
                        mv[:, 0:1],
                        istd[:],
                        ALU.subtract,
                        ALU.mult,
                    )
                xc.append(xc_t)

            xnT = singles.tile([128, KO, RL], BF16, tag="xnT")
            for ko in range(KO):
                ptr = psum.tile([128, 4, 128], BF16, tag="pm")
                for t in range(4):
                    nc.tensor.transpose(
                        ptr[:, t, :], xc[t][:, ko * 128 : (ko + 1) * 128], ident[:]
                    )
                nc.vector.tensor_scalar(
                    xnT[:, ko, :],
                    ptr.rearrange("p a b -> p (a b)"),
                    gamma_sb[:, ko : ko + 1],
                    beta_sb[:, ko : ko + 1],
                    ALU.mult,
                    ALU.add,
                )

            # ---- Phase 2: K projection (own rows, all heads) -> bounce ----
            for hp in range(HP):
                pm = psum.tile([128, RL], F32, tag="pm", name="pm_k")
                for ko in range(KO):
                    nc.tensor.matmul(
                        pm[:],
                        lhsT=wk_b[:, ko, hp * 128 : (hp + 1) * 128],
                        rhs=xnT[:, ko, :],
                        start=(ko == 0),
                        stop=(ko == KO - 1),
                    )
                kst = small.tile([128, RL], BF16, tag="kst", name="kst")
                nc.vector.tensor_copy(out=kst[:], in_=pm[:])
                nc.scalar.dma_start(
                    out=k_bounce[hp * 128 : (hp + 1) * 128, :], in_=kst[:]
                )

            # ---- AllGather-K doorbell ----
            nc.gpsimd.collective_compute(
                "AllGather",
                ALU.bypass,
                ins=[k_bounce[:]],
                outs=[k_gath[:]],
                replica_groups=GROUPS,
            )

            # ---- Phase 3: V projection (own rows, natural layout) ----
            for rt in range(4):
                for nh in range(2):
                    pv = psum.tile([128, RL], F32, tag="pm", name="pm_v")
                    for ko in range(KO):
                        nc.tensor.matmul(
                            pv[:],
                            lhsT=xnT[:, ko, rt * 128 : (rt + 1) * 128],
                            rhs=wv_b[:, ko, nh * 512 : (nh + 1) * 512],
                            start=(ko == 0),
                            stop=(ko == KO - 1),
                        )
                    vst = small.tile([128, RL], BF16, tag="vst", name="vst")
                    nc.vector.tensor_copy(out=vst[:], in_=pv[:])
                    nc.scalar.dma_start(
                        out=v_bounce[
                            rt * 128 : (rt + 1) * 128, nh * 512 : (nh + 1) * 512
                        ],
                        in_=vst[:],
                    )

            # ---- AllGather-V doorbell ----
            nc.gpsimd.collective_compute(
                "AllGather",
                ALU.bypass,
                ins=[v_bounce[:]],
                outs=[v_gath[:]],
                replica_groups=GROUPS,
            )

            # ---- Phase 4: Q projection (local; overlaps AG-K) ----
            qT = singles.tile([128, HP, RL], BF16, tag="qT")
            for hp in range(HP):
                pm = psum.tile([128, RL], F32, tag="pm", name="pm_q")
                for ko in range(KO):
                    nc.tensor.matmul(
                        pm[:],
                        lhsT=wq_b[:, ko, hp * 128 : (hp + 1) * 128],
                        rhs=xnT[:, ko, :],
                        start=(ko == 0),
                        stop=(ko == KO - 1),
                    )
                nc.vector.tensor_copy(out=qT[:, hp, :], in_=pm[:])

            # wo load+cast mid-attention (gpsimd DMA, vector cast)
            def load_wo():
                for c in range(KO):
                    wf = wstage.tile([128, DIM], F32, tag="wstage", name="wf_o")
                    nc.gpsimd.dma_start(
                        out=wf[:], in_=wo_ext[c * 128 : (c + 1) * 128, :]
                    )
                    nc.vector.tensor_copy(out=wo_b[:, c, :], in_=wf[:])

            # ---- Phase 5: attention per head-pair ----
            aoT = singles.tile([128, HP, RL], BF16, tag="aoT")

            for hp in range(HP):
                kT = kvpool.tile([128, G, RL], BF16, tag="kT", name="kT")
                nc.gpsimd.dma_start(
                    out=kT[:],
                    in_=bass.AP(
                        tensor=k_gath,
                        offset=hp * 128 * RL,
                        ap=[[RL, 128], [DIM * RL, G], [1, RL]],
                    ),
                )
                v3 = kvpool.tile([128, NJT, 2, 2 * DH], BF16, tag="v3", name="v3")
                nc.vector.memset(v3[:, :, :, DH : 2 * DH], 1.0)
                for h in range(2):
                    nc.gpsimd.dma_start(
                        out=v3[:, :, h, 0:DH],
                        in_=bass.AP(
                            tensor=v_gath,
                            offset=hp * 128 + h * DH,
                            ap=[[DIM, 128], [128 * DIM, NJT], [1, DH]],
                        ),
                    )
                if hp == 0:
                    load_wo()

                avT0 = psum.tile([128, RL], F32, tag="avT0", bufs=1, name="avT0")
                avT1 = psum.tile([128, RL], F32, tag="avT1", bufs=1, name="avT1")
                avTs = (avT0, avT1)
                pend = []

                def flush_one():
                    jt_, e_ = pend.pop(0)
                    for h_ in range(2):
                        nc.tensor.matmul(
                            avTs[h_][:],
                            lhsT=v3[:, jt_, h_, :],
                            rhs=e_[:, h_, :],
                            start=(jt_ == 0),
                            stop=(jt_ == NJT - 1),
                        )

                lag = 12 if hp == 0 else 2
                for jt in range(NJT):
                    sc = psum.tile([128, 2, RL], F32, tag="sc", name="sc")
                    for h in range(2):
                        nc.tensor.matmul(
                            sc[:, h, :],
                            lhsT=kT[
                                DH * h : DH * (h + 1),
                                jt // 4,
                                (jt % 4) * 128 : (jt % 4 + 1) * 128,
                            ],
                            rhs=qT[DH * h : DH * (h + 1), hp, :],
                            start=True,
                            stop=True,
                        )
                    e = epool.tile([128, 2, RL], BF16, tag="etile", name="e")
                    nc.scalar.activation(
                        out=e.rearrange("p a b -> p (a b)"),
                        in_=sc.rearrange("p a b -> p (a b)"),
                        func=AF.Exp,
                        scale=SCALE,
                    )
                    pend.append((jt, e))
                    while len(pend) > lag:
                        flush_one()
                while pend:
                    flush_one()

                # drain + normalize (denominators in partitions 64-127)
                for h in range(2):
                    dr = small.tile([128, RL], F32, tag="drain", name="drain")
                    nc.vector.tensor_copy(out=dr[:], in_=avTs[h][:])
                    rec = small.tile([DH, RL], F32, tag="rec", name="rec")
                    nc.vector.reciprocal(out=rec[:], in_=dr[DH : 2 * DH, :])
                    nc.vector.tensor_tensor(
                        out=aoT[h * DH : (h + 1) * DH, hp, :],
                        in0=dr[0:DH, :],
                        in1=rec[:],
                        op=ALU.mult,
                    )

            # ---- Phase 6: out-projection + bias on own rows ----
            for rt in range(4):
                for nh in range(2):
                    po = psum.tile([128, RL], F32, tag="pm", name="pm_o")
                    for hp in range(HP):
                        nc.tensor.matmul(
                            po[:],
                            lhsT=aoT[:, hp, rt * 128 : (rt + 1) * 128],
                            rhs=wo_b[:, hp, nh * 512 : (nh + 1) * 512],
                            start=(hp == 0),
                            stop=(hp == HP - 1),
                        )
                    o_sb = temps.tile([128, RL], F32, tag="osb")
                    nc.vector.tensor_tensor(
                        out=o_sb[:],
                        in0=po[:],
                        in1=bias_sb[:, nh * 512 : (nh + 1) * 512],
                        op=ALU.add,
                    )
                    nc.gpsimd.dma_start(
                        out=out_ext[
                            rt * 128 : (rt + 1) * 128, nh * 512 : (nh + 1) * 512
                        ],
                        in_=o_sb[:],
                    )

    nc.compile()
    return nc


_NC_CACHE = None


def _get_nc():
    global _NC_CACHE
    if _NC_CACHE is None:
        _NC_CACHE = build_nc()
    return _NC_CACHE


def _shard_inputs(x, w_q, w_kv, w_out, b_out, gamma, beta):
    xr = np.ascontiguousarray(x.reshape(R, DIM))
    wk = np.ascontiguousarray(w_kv[:, :DIM])
    wv = np.ascontiguousarray(w_kv[:, DIM:])
    in_maps = []
    for c in range(N_CORES):
        in_maps.append(
            {
                "x": np.ascontiguousarray(xr[RL * c : RL * (c + 1)]),
                "wq": np.ascontiguousarray(w_q),
                "wk": wk,
                "wv": wv,
                "wo": np.ascontiguousarray(w_out),
                "gamma": np.ascontiguousarray(gamma),
                "beta": np.ascontiguousarray(beta),
                "bias": np.ascontiguousarray(b_out),
            }
        )
    return in_maps


def run_sharded(x, w_q, w_kv, w_out, b_out, gamma, beta, trace=False, **trace_kwargs):
    nc = _get_nc()
    in_maps = _shard_inputs(
        np.asarray(x, np.float32),
        np.asarray(w_q, np.float32),
        np.asarray(w_kv, np.float32),
        np.asarray(w_out, np.float32),
        np.asarray(b_out, np.float32),
        np.asarray(gamma, np.float32),
        np.asarray(beta, np.float32),
    )
    res = run_bass_kernel_spmd(
        nc, in_maps, core_ids=list(range(N_CORES)), trace=trace, **trace_kwargs
    )
    out = np.concatenate([res.results[c]["out"] for c in range(N_CORES)], axis=0)
    return out.reshape(2, N, DIM), res


def kernel(x, w_q, w_kv, w_out, b_out, gamma, beta):
    out, _ = run_sharded(x, w_q, w_kv, w_out, b_out, gamma, beta, trace=False)
    return out
